# revision 1
# baseline (speedup 1.0000x reference)
"""Windowed local self-attention (CrossAttention module with the context-
overwrite bug faithfully reproduced) on 8 Trainium2 NeuronCores.

Full-input contract: kernel(**inputs) takes the unsharded tensors and
returns the full (4, 4096, 1024) output. Internally the 64 independent
windows of 256 tokens are data-parallel sharded 8-per-core; the four
projection weights are broadcast to every core. No collectives needed.

Per-core pipeline (window = 256 tokens, H=16 heads, DH=64):
  X  --PE transpose-->  XT [d, i]
  qT = Wq.T @ X.T   (lhsT=Wq tiles,  rhs=XT)          [o, i]
  kT = Wk.T @ X.T                                      [o, i]
  v  = X @ Wv       (lhsT=XT tiles,  rhs=Wv)           [j, o]
  per head h:
    simT = kT_h.T-free @ qT_h   -> [j, i] in PSUM     (j on partitions)
    es   = exp(0.125 * simT)    (ACT, PSUM->SBUF)
    S    = ones[j,64].T @ es    -> [64, i] broadcast row-sums (PE)
    rS   = 1/S                  (DVE reciprocal)
    o2u  = v_h.T-free @ es      -> [d, i] in PSUM      (AV matmul)
    o2T  = o2u * rS             (DVE, writes stacked [o, i] SBUF)
  Y = o2T.T @ Wo       (lhsT=o2T tiles, rhs=Wo; zero bias added host-side)
All matmul operands are bitcast to float32r: full fp32 bits, 1 cycle/row
on the PE at moving free-dim >= 256 (vs 4 cycles/row for plain float32).
"""

import numpy as np

import concourse.bass as bass
import concourse.mybir as mybir
import concourse.tile as tile
from concourse import bacc, bass_utils
from concourse.bass_interp import get_hw_module
from concourse.masks import make_identity

H = 16
DH = 64
WIN = 256
D = 1024
B = 4
N = 4096
N_CORES = 8
N_WIN_TOTAL = B * N // WIN          # 64
N_WIN = N_WIN_TOTAL // N_CORES      # 8 windows per core
TOK = N_WIN * WIN                   # 2048 token rows per core
SCALE = DH ** -0.5

F32 = mybir.dt.float32
F32R = mybir.dt.float32r


def _r(ap):
    return ap.bitcast(F32R)


def _body(tc, xq, wq, wk, wv, wo, out, n_win):
    nc = tc.nc
    from contextlib import ExitStack

    with ExitStack() as ctx:
        singles = ctx.enter_context(tc.tile_pool(name="singles", bufs=1))
        xpool = ctx.enter_context(tc.tile_pool(name="xpool", bufs=2))
        acts = ctx.enter_context(tc.tile_pool(name="acts", bufs=1))
        heads = ctx.enter_context(tc.tile_pool(name="heads", bufs=2))
        ypool = ctx.enter_context(tc.tile_pool(name="ypool", bufs=2))
        psA = ctx.enter_context(tc.tile_pool(name="psA", bufs=2, space="PSUM"))
        psS = ctx.enter_context(tc.tile_pool(name="psS", bufs=2, space="PSUM"))
        psV = ctx.enter_context(tc.tile_pool(name="psV", bufs=2, space="PSUM"))

        # ---- constants / weights (resident all kernel) ----
        ident_f = singles.tile([128, 128], F32)
        make_identity(nc, ident_f[:])
        ident = singles.tile([128, 128], F32R)
        nc.vector.tensor_copy(ident[:], ident_f[:])
        ones_f = singles.tile([128, 64], F32)
        nc.gpsimd.memset(ones_f[:], 1.0)
        ones64 = singles.tile([128, 64], F32R)
        nc.vector.tensor_copy(ones64[:], ones_f[:])

        # first window's X before the big weight DMAs so transposes start early
        x_first = [xpool.tile([128, D], F32R, tag="x", name=f"x0_{i}") for i in range(2)]
        for tt in range(2):
            nc.sync.dma_start(x_first[tt][:], xq[tt * 128:(tt + 1) * 128, :])

        wsb = {}
        for name, w in (("wq", wq), ("wk", wk), ("wv", wv), ("wo", wo)):
            t = singles.tile([128, 8 * D], F32R, tag=name, name=f"sb_{name}")
            for kt in range(8):
                nc.sync.dma_start(
                    t[:, kt * D:(kt + 1) * D], w[kt * 128:(kt + 1) * 128, :]
                )
            wsb[name] = t

        def emit_transposes(w, x_sb, xt):
            for dt_ in range(8):
                for tt in range(2):
                    pt = psA.tile([128, 128], F32R, tag="acc", name=f"pt_{w}_{dt_}_{tt}")
                    nc.tensor.transpose(
                        pt[:], x_sb[tt][:, dt_ * 128:(dt_ + 1) * 128], ident[:]
                    )
                    nc.vector.tensor_copy(
                        xt[:, dt_ * WIN + tt * 128:dt_ * WIN + tt * 128 + 128], pt[:]
                    )

        def emit_y_group(w, o2T, it, ec):
            row0 = w * WIN
            py = psA.tile([128, 512], F32, tag="acc", name=f"py_{w}_{it}_{ec}")
            for kt2 in range(8):
                nc.tensor.matmul(
                    py[:],
                    o2T[:, kt2 * WIN + it * 128:kt2 * WIN + (it + 1) * 128],
                    wsb["wo"][:, kt2 * D + ec * 512:kt2 * D + (ec + 1) * 512],
                    start=(kt2 == 0),
                    stop=(kt2 == 7),
                )
            y_sb = ypool.tile([128, 512], F32, tag="y", name=f"y_{w}_{it}_{ec}")
            nc.vector.tensor_copy(y_sb[:], py[:])
            nc.sync.dma_start(
                out[row0 + it * 128:row0 + (it + 1) * 128, ec * 512:(ec + 1) * 512],
                y_sb[:],
            )

        prev = None  # (o2T of previous window)
        for w in range(n_win):
            row0 = w * WIN
            if w == 0:
                x_sb = x_first
            else:
                x_sb = [xpool.tile([128, D], F32R, tag="x", name=f"x_{w}_{i}") for i in range(2)]
                for tt in range(2):
                    nc.sync.dma_start(
                        x_sb[tt][:], xq[row0 + tt * 128:row0 + (tt + 1) * 128, :]
                    )

            xt = acts.tile([128, 8 * WIN], F32R, tag="xt", name=f"xt_{w}")
            if prev is None:
                emit_transposes(w, x_sb, xt)
            else:
                # interleave: 4 transposes, then one Y group of previous window
                for chunk in range(4):
                    for dt_ in range(2 * chunk, 2 * chunk + 2):
                        for tt in range(2):
                            pt = psA.tile([128, 128], F32R, tag="acc",
                                          name=f"pt_{w}_{dt_}_{tt}")
                            nc.tensor.transpose(
                                pt[:], x_sb[tt][:, dt_ * 128:(dt_ + 1) * 128], ident[:]
                            )
                            nc.vector.tensor_copy(
                                xt[:, dt_ * WIN + tt * 128:dt_ * WIN + tt * 128 + 128],
                                pt[:],
                            )
                    emit_y_group(w - 1, prev, chunk // 2, chunk % 2)

            # ---- qT, kT [128, 2048] ----
            proj = {}
            for pname, wname in (("qT", "wq"), ("kT", "wk")):
                dst = acts.tile([128, 8 * WIN], F32R, tag=pname, name=f"{pname}_{w}")
                wtile = wsb[wname]
                for ot in range(8):
                    pq = psA.tile([128, WIN], F32, tag="acc", name=f"pq_{w}_{pname}_{ot}")
                    for kt in range(8):
                        nc.tensor.matmul(
                            pq[:],
                            wtile[:, kt * D + ot * 128:kt * D + (ot + 1) * 128],
                            xt[:, kt * WIN:(kt + 1) * WIN],
                            start=(kt == 0),
                            stop=(kt == 7),
                        )
                    nc.vector.tensor_copy(dst[:, ot * WIN:(ot + 1) * WIN], pq[:])
                proj[pname] = dst
            qT, kT = proj["qT"], proj["kT"]

            # ---- v natural [128 j, 2048] ----
            v_sb = acts.tile([128, 2 * D], F32R, tag="v", name=f"v_{w}")
            for jt in range(2):
                for oc in range(2):
                    pv = psA.tile([128, 512], F32, tag="acc", name=f"pv_{w}_{jt}_{oc}")
                    for kt in range(8):
                        nc.tensor.matmul(
                            pv[:],
                            xt[:, kt * WIN + jt * 128:kt * WIN + (jt + 1) * 128],
                            wsb["wv"][:, kt * D + oc * 512:kt * D + (oc + 1) * 512],
                            start=(kt == 0),
                            stop=(kt == 7),
                        )
                    nc.vector.tensor_copy(
                        v_sb[:, jt * D + oc * 512:jt * D + (oc + 1) * 512], pv[:]
                    )

            # ---- attention: head pairs, software-pipelined ----
            o2T = acts.tile([128, 8 * WIN], F32R, tag="o2T", name=f"o2T_{w}")

            es_t = [None] * H

            def emit_sim(h):
                prow = (h % 2) * 64
                ocol = (h // 2) * WIN
                qh = qT[prow:prow + 64, ocol:ocol + WIN]
                kh = kT[prow:prow + 64, ocol:ocol + WIN]
                ps_sim = psS.tile([128, 512], F32, tag="sim", name=f"sim_{w}_{h}")
                for jt in range(2):
                    nc.tensor.matmul(
                        ps_sim[:, jt * WIN:(jt + 1) * WIN],
                        kh[:, jt * 128:(jt + 1) * 128],
                        qh,
                        start=True,
                        stop=True,
                    )
                e = heads.tile([128, 512], F32R, tag="es", name=f"es_{w}_{h}")
                nc.scalar.activation(
                    e[:], ps_sim[:], mybir.ActivationFunctionType.Exp, scale=SCALE
                )
                es_t[h] = e

            def emit_pair(p):
                for h in (2 * p, 2 * p + 1):
                    s_ps = psV.tile([64, WIN], F32, tag="s", bufs=2,
                                    name=f"s_{w}_{h}")
                    av_ps = psV.tile([64, WIN], F32, tag="av", bufs=2,
                                     name=f"av_{w}_{h}")
                    for jt in range(2):
                        nc.tensor.matmul(
                            s_ps[:],
                            ones64[:, 0:64],
                            es_t[h][:, jt * WIN:(jt + 1) * WIN],
                            start=(jt == 0),
                            stop=(jt == 1),
                        )
                    for jt in range(2):
                        nc.tensor.matmul(
                            av_ps[:],
                            v_sb[:, jt * D + h * DH:jt * D + (h + 1) * DH],
                            es_t[h][:, jt * WIN:(jt + 1) * WIN],
                            start=(jt == 0),
                            stop=(jt == 1),
                        )
                    s_sb = heads.tile([64, WIN], F32, tag="s_sb",
                                      name=f"ssb_{w}_{h}")
                    nc.vector.tensor_copy(s_sb[:], s_ps[:])
                    rs = heads.tile([64, WIN], F32, tag="rs", name=f"rs_{w}_{h}")
                    nc.vector.reciprocal_approx_fast(rs[:], s_sb[:])
                    r0 = (h % 2) * 64
                    nc.vector.tensor_mul(
                        o2T[r0:r0 + 64, p * WIN:(p + 1) * WIN], av_ps[:], rs[:]
                    )
                    es_t[h] = None

            emit_sim(0)
            emit_sim(1)
            for p in range(1, 8):
                emit_sim(2 * p)
                emit_sim(2 * p + 1)
                emit_pair(p - 1)
            emit_pair(7)

            prev = o2T

        for chunk in range(4):
            emit_y_group(n_win - 1, prev, chunk // 2, chunk % 2)


_CACHE = {}


def _build(n_win=N_WIN):
    key = n_win
    if key in _CACHE:
        return _CACHE[key]
    tok = n_win * WIN
    nc = bacc.Bacc(
        "TRN2", target_bir_lowering=False, debug=False, num_devices=N_CORES
    )
    xq = nc.dram_tensor("xq", [tok, D], F32R, kind="ExternalInput").ap()
    wq = nc.dram_tensor("Wq", [D, D], F32R, kind="ExternalInput").ap()
    wk = nc.dram_tensor("Wk", [D, D], F32R, kind="ExternalInput").ap()
    wv = nc.dram_tensor("Wv", [D, D], F32R, kind="ExternalInput").ap()
    wo = nc.dram_tensor("Wo", [D, D], F32R, kind="ExternalInput").ap()
    out = nc.dram_tensor("out", [tok, D], F32, kind="ExternalOutput").ap()
    with tile.TileContext(nc) as tc:
        _body(tc, xq, wq, wk, wv, wo, out, n_win)
    nc.compile()
    nc.m = get_hw_module(nc.m)
    _CACHE[key] = nc
    return nc


def run(query, Wq, Wk, Wv, Wo, bo, n_win=N_WIN, **spmd_kwargs):
    nc = _build(n_win)
    tok = n_win * WIN
    q2 = np.ascontiguousarray(np.asarray(query, dtype=np.float32).reshape(-1, D))
    weights = {
        "Wq": np.ascontiguousarray(np.asarray(Wq, np.float32)),
        "Wk": np.ascontiguousarray(np.asarray(Wk, np.float32)),
        "Wv": np.ascontiguousarray(np.asarray(Wv, np.float32)),
        "Wo": np.ascontiguousarray(np.asarray(Wo, np.float32)),
    }
    in_maps = []
    for c in range(N_CORES):
        m = {"xq": q2[c * TOK:c * TOK + tok]}
        m.update(weights)
        in_maps.append(m)
    res = bass_utils.run_bass_kernel_spmd(
        nc, in_maps, core_ids=list(range(N_CORES)), **spmd_kwargs
    )
    outs = [res.results[c]["out"] for c in range(N_CORES)]
    return outs, res


def kernel(query, context, Wq, Wk, Wv, Wo, bo):
    outs, _ = run(query, Wq, Wk, Wv, Wo, bo)
    y = np.concatenate(outs, axis=0).reshape(B, N, D)
    bo = np.asarray(bo, np.float32)
    if bo.any():
        y = y + bo  # bias is structurally zero for this problem; host-add keeps exactness
    return y.astype(np.float32)



# revision 6
# speedup vs baseline: 1.5783x; 1.5783x over previous
"""Windowed local self-attention (CrossAttention with the context-overwrite
bug reproduced) on 8 Trainium2 NeuronCores — bf16 rewrite.

Full-input contract: kernel(**inputs) takes unsharded tensors, returns the
full (4, 4096, 1024) fp32 output. The 64 independent 256-token windows are
data-parallel sharded 8 per core; all four projection weights are broadcast.
No collectives.

Key structure (vs the fp32r baseline):
  * All matmul operands are bf16 (tolerance 2e-2; measured ~1e-3). bf16
    enables compiler-automatic FWL (2x faster LDWEIGHTS, hidden by the PE's
    64-deep reorder window).
  * X^T is produced by the DMA XBAR transpose (16-bit dtype) straight from
    HBM — no PE transposes, no DVE fixup copies.
  * Windows are processed in pairs so the projection matmuls stream N=512
    moving columns per instruction (half the instruction count).
  * Attention uses PE tile_position concurrency: sim row-tiles the two heads
    of an o-tile (K=64 each at row groups 0/64), S (ones-matmul row sums) and
    AV col-tile the same two heads (M=64 at col groups 0/64). Auto-derived
    from base partitions.
  * Software pipeline over pairs: iteration p issues proj(p) interleaved
    with attention(p-1) and the output projection Y(p-2), keeping the PE
    dense (HAM stays at K=8/8) while exp() runs on the ACT engine.

Per-pair layouts (i = pair-local token 0..511, ws = i//256):
  xt  [128, 8*512]  bf16  block kt: X^T[d in kt-tile, i]
  qT  [128, 8*512]  bf16  block ot: (XWq)^T[o in ot-tile, i]   (kT same)
  v   [128, 4*1024] bf16  block jj=(ws,jt): V[j in jj-tile, o]
  o2T [128, 8*512]  bf16  block ot: normalized attention output^T
"""

import numpy as np
import ml_dtypes

import concourse.bass as bass
import concourse.mybir as mybir
import concourse.tile as tile
from concourse import bacc, bass_utils
from concourse.bass_interp import get_hw_module

H = 16
DH = 64
WIN = 256
D = 1024
B = 4
N = 4096
N_CORES = 8
N_WIN_TOTAL = B * N // WIN          # 64
N_WIN = N_WIN_TOTAL // N_CORES      # 8 windows per core
N_PAIR = N_WIN // 2                 # 4 window-pairs per core
TOK = N_WIN * WIN                   # 2048 token rows per core
PTOK = 2 * WIN                      # 512 tokens per pair
SCALE = DH ** -0.5

F32 = mybir.dt.float32
BF16 = mybir.dt.bfloat16


def _body(tc, xq, wq, wk, wv, wo, out, n_pair):
    nc = tc.nc
    from contextlib import ExitStack

    with ExitStack() as ctx:
        singles = ctx.enter_context(tc.tile_pool(name="singles", bufs=1))
        xpool = ctx.enter_context(tc.tile_pool(name="xpool", bufs=2))
        acts = ctx.enter_context(tc.tile_pool(name="acts", bufs=2))
        heads = ctx.enter_context(tc.tile_pool(name="heads", bufs=4))
        ypool = ctx.enter_context(tc.tile_pool(name="ypool", bufs=2))
        psP = ctx.enter_context(tc.tile_pool(name="psP", bufs=3, space="PSUM"))
        psS = ctx.enter_context(tc.tile_pool(name="psS", bufs=2, space="PSUM"))
        psV = ctx.enter_context(tc.tile_pool(name="psV", bufs=2, space="PSUM"))

        # ---- xt of pair 0 first so its XBAR transposes lead the DMA queue
        def alloc_xt(p):
            t = xpool.tile([128, 8 * PTOK], BF16, tag="xt", name=f"xt_{p}")
            for kt in range(8):
                for ws in range(2):
                    r0 = p * PTOK + ws * WIN
                    nc.sync.dma_start(
                        t[:, kt * PTOK + ws * WIN:kt * PTOK + (ws + 1) * WIN],
                        xq[r0:r0 + WIN, kt * 128:(kt + 1) * 128],
                        transpose=True,
                    )
            return t

        xt_cur = alloc_xt(0)

        # ---- constants / weights (resident all kernel) ----
        ones_f = singles.tile([128, 64], F32)
        nc.gpsimd.memset(ones_f[:], 1.0)
        ones64 = singles.tile([128, 64], BF16)
        nc.vector.tensor_copy(ones64[:], ones_f[:])

        wsb = {}
        for name, w in (("wq", wq), ("wk", wk), ("wv", wv), ("wo", wo)):
            t = singles.tile([128, 8 * D], BF16, tag=name, name=f"sb_{name}")
            for kt in range(8):
                nc.sync.dma_start(
                    t[:, kt * D:(kt + 1) * D], w[kt * 128:(kt + 1) * 128, :]
                )
            wsb[name] = t

        # ================= emission helpers =================

        def emit_qkT_unit(p, dst, wname, ot):
            pq = psP.tile([128, PTOK], F32, tag="acc", name=f"pq_{p}_{wname}_{ot}")
            wtile = wsb[wname]
            for kt in range(8):
                nc.tensor.matmul(
                    pq[:],
                    wtile[:, kt * D + ot * 128:kt * D + (ot + 1) * 128],
                    xts[p][:, kt * PTOK:(kt + 1) * PTOK],
                    start=(kt == 0),
                    stop=(kt == 7),
                )
            nc.vector.tensor_copy(dst[:, ot * PTOK:(ot + 1) * PTOK], pq[:])

        def emit_v_unit(p, v_sb, jj, oc):
            pv = psP.tile([128, 512], F32, tag="acc", name=f"pv_{p}_{jj}_{oc}")
            for kt in range(8):
                nc.tensor.matmul(
                    pv[:],
                    xts[p][:, kt * PTOK + jj * 128:kt * PTOK + (jj + 1) * 128],
                    wsb["wv"][:, kt * D + oc * 512:kt * D + (oc + 1) * 512],
                    start=(kt == 0),
                    stop=(kt == 7),
                )
            nc.vector.tensor_copy(v_sb[:, jj * D + oc * 512:jj * D + (oc + 1) * 512], pv[:])

        def emit_sim(st, qT, kT, ws, pp):
            """sim for head pair (2pp, 2pp+1) of window ws; row-tiled."""
            col0 = pp * PTOK + ws * WIN
            es_pair = []
            ps_pair = []
            for hh in range(2):          # hh = h % 2, row group hh*64
                r0 = hh * 64
                ps_h = psS.tile([128, 512], F32, tag="sim",
                                name=f"sim_{st}_{hh}")
                ps_pair.append(ps_h)
            for jt in range(2):
                for hh in range(2):
                    r0 = hh * 64
                    nc.tensor.matmul(
                        ps_pair[hh][:, jt * WIN:(jt + 1) * WIN],
                        kT[r0:r0 + 64, col0 + jt * 128:col0 + (jt + 1) * 128],
                        qT[r0:r0 + 64, col0:col0 + WIN],
                        start=True,
                        stop=True,
                    )
            for hh in range(2):
                e = heads.tile([128, 512], BF16, tag="es", name=f"es_{st}_{hh}")
                nc.scalar.activation(
                    e[:], ps_pair[hh][:], mybir.ActivationFunctionType.Exp,
                    scale=SCALE,
                )
                es_pair.append(e)
            return es_pair

        def emit_sav(st, es_pair, v_sb, o2T, ws, pp):
            """row-sum S + AV for head pair; col-tiled; writes o2T block."""
            col0 = pp * PTOK + ws * WIN
            sav = psV.tile([128, 2 * WIN], F32, tag="sav", name=f"sav_{st}")
            s_ps = sav[:, 0:WIN]
            av_ps = sav[:, WIN:2 * WIN]
            for jt in range(2):
                for hh in range(2):
                    nc.tensor.matmul(
                        s_ps[hh * 64:(hh + 1) * 64, :],
                        ones64[:],
                        es_pair[hh][:, jt * WIN:(jt + 1) * WIN],
                        start=(jt == 0),
                        stop=(jt == 1),
                        skip_group_check=True,
                    )
            for jt in range(2):
                jj = ws * 2 + jt
                for hh in range(2):
                    h = 2 * pp + hh
                    nc.tensor.matmul(
                        av_ps[hh * 64:(hh + 1) * 64, :],
                        v_sb[:, jj * D + h * DH:jj * D + (h + 1) * DH],
                        es_pair[hh][:, jt * WIN:(jt + 1) * WIN],
                        start=(jt == 0),
                        stop=(jt == 1),
                        skip_group_check=True,
                    )
            rs = heads.tile([128, WIN], F32, tag="rs", name=f"rs_{st}", bufs=2)
            nc.vector.reciprocal_approx_fast(rs[:], s_ps[:])
            nc.vector.tensor_mul(o2T[:, col0:col0 + WIN], av_ps[:], rs[:])

        def emit_y_group(p, o2T, it, ec):
            py = psP.tile([128, 512], F32, tag="acc", name=f"py_{p}_{it}_{ec}")
            for ot in range(8):
                nc.tensor.matmul(
                    py[:],
                    o2T[:, ot * PTOK + it * 128:ot * PTOK + (it + 1) * 128],
                    wsb["wo"][:, ot * D + ec * 512:ot * D + (ec + 1) * 512],
                    start=(ot == 0),
                    stop=(ot == 7),
                )
            y_sb = ypool.tile([128, 512], F32, tag="y", name=f"y_{p}_{it}_{ec}")
            nc.scalar.copy(y_sb[:], py[:])
            r0 = p * PTOK + it * 128
            nc.sync.dma_start(out[r0:r0 + 128, ec * 512:(ec + 1) * 512], y_sb[:])

        # ================= software pipeline =================

        xts = {0: xt_cur}
        pair_tiles = {}   # p -> (qT, kT, v_sb, o2T)
        ATT_STEPS = [(ws, pp) for ws in range(2) for pp in range(8)]

        def attn_step_sim(p, st):
            ws, pp = ATT_STEPS[st]
            qT, kT, v_sb, o2T = pair_tiles[p]
            return emit_sim(f"{p}_{st}", qT, kT, ws, pp)

        def attn_step_sav(p, st, es_pair):
            ws, pp = ATT_STEPS[st]
            qT, kT, v_sb, o2T = pair_tiles[p]
            emit_sav(f"{p}_{st}", es_pair, v_sb, o2T, ws, pp)

        for p in range(n_pair):
            if p + 1 < n_pair:
                xts[p + 1] = alloc_xt(p + 1)

            qT = acts.tile([128, 8 * PTOK], BF16, tag="qT", name=f"qT_{p}")
            kT = acts.tile([128, 8 * PTOK], BF16, tag="kT", name=f"kT_{p}")
            v_sb = acts.tile([128, 4 * D], BF16, tag="v", name=f"v_{p}")
            o2T = acts.tile([128, 8 * PTOK], BF16, tag="o2T", name=f"o2T_{p}")
            pair_tiles[p] = (qT, kT, v_sb, o2T)

            proj_units = (
                [lambda ot=ot: emit_qkT_unit(p, qT, "wq", ot) for ot in range(8)]
                + [lambda ot=ot: emit_qkT_unit(p, kT, "wk", ot) for ot in range(8)]
                + [lambda jj=jj, oc=oc: emit_v_unit(p, v_sb, jj, oc)
                   for jj in range(4) for oc in range(2)]
            )
            y_units = []
            if p >= 2:
                o2T_y = pair_tiles[p - 2][3]
                y_units = [
                    lambda it=it, ec=ec: emit_y_group(p - 2, o2T_y, it, ec)
                    for it in range(4) for ec in range(2)
                ]

            if p == 0:
                for u in proj_units:
                    u()
            else:
                # interleave: 16 attention steps of pair p-1, one-step
                # sim->sav delay, filler = proj units of p + Y units of p-2
                filler = proj_units + y_units
                fi = 0
                pending = None
                for st in range(16):
                    es_pair = attn_step_sim(p - 1, st)
                    if fi < len(filler):
                        filler[fi](); fi += 1
                    if pending is not None:
                        attn_step_sav(p - 1, pending[0], pending[1])
                    pending = (st, es_pair)
                    if fi < len(filler):
                        filler[fi](); fi += 1
                attn_step_sav(p - 1, pending[0], pending[1])
                while fi < len(filler):
                    filler[fi](); fi += 1

        # ---- drain: attention of last pair + Y of pairs n-2, n-1 ----
        last = n_pair - 1
        filler = []
        if n_pair >= 2:
            o2T_y = pair_tiles[n_pair - 2][3]
            filler = [
                lambda it=it, ec=ec: emit_y_group(n_pair - 2, o2T_y, it, ec)
                for it in range(4) for ec in range(2)
            ]
        fi = 0
        pending = None
        late_y = []
        o2T_last = pair_tiles[last][3]
        for st in range(16):
            es_pair = attn_step_sim(last, st)
            if fi < len(filler):
                filler[fi](); fi += 1
            if pending is not None:
                attn_step_sav(last, pending[0], pending[1])
            pending = (st, es_pair)
            if st == 9:
                # window 0 of the last pair fully normalized after step 7's
                # sav (emitted during st=8); its Y groups are ready now
                late_y = [
                    lambda it=it, ec=ec: emit_y_group(last, o2T_last, it, ec)
                    for it in range(2) for ec in range(2)
                ]
            if late_y:
                late_y.pop(0)()
            elif fi < len(filler):
                filler[fi](); fi += 1
        attn_step_sav(last, pending[0], pending[1])
        while fi < len(filler):
            filler[fi](); fi += 1
        for it in range(2, 4):
            for ec in range(2):
                emit_y_group(last, o2T_last, it, ec)


_CACHE = {}


def _build(n_pair=N_PAIR):
    key = n_pair
    if key in _CACHE:
        return _CACHE[key]
    tok = n_pair * PTOK
    nc = bacc.Bacc(
        "TRN2", target_bir_lowering=False, debug=False, num_devices=N_CORES
    )
    xq = nc.dram_tensor("xq", [tok, D], BF16, kind="ExternalInput").ap()
    wq = nc.dram_tensor("Wq", [D, D], BF16, kind="ExternalInput").ap()
    wk = nc.dram_tensor("Wk", [D, D], BF16, kind="ExternalInput").ap()
    wv = nc.dram_tensor("Wv", [D, D], BF16, kind="ExternalInput").ap()
    wo = nc.dram_tensor("Wo", [D, D], BF16, kind="ExternalInput").ap()
    out = nc.dram_tensor("out", [tok, D], F32, kind="ExternalOutput").ap()
    with tile.TileContext(nc) as tc:
        _body(tc, xq, wq, wk, wv, wo, out, n_pair)
    nc.compile()
    nc.m = get_hw_module(nc.m)
    _CACHE[key] = nc
    return nc


def run(query, Wq, Wk, Wv, Wo, bo, n_pair=N_PAIR, **spmd_kwargs):
    nc = _build(n_pair)
    tok = n_pair * PTOK
    q2 = np.ascontiguousarray(
        np.asarray(query, dtype=np.float32).reshape(-1, D)
    ).astype(ml_dtypes.bfloat16)
    weights = {
        "Wq": np.asarray(Wq, np.float32).astype(ml_dtypes.bfloat16),
        "Wk": np.asarray(Wk, np.float32).astype(ml_dtypes.bfloat16),
        "Wv": np.asarray(Wv, np.float32).astype(ml_dtypes.bfloat16),
        "Wo": np.asarray(Wo, np.float32).astype(ml_dtypes.bfloat16),
    }
    in_maps = []
    for c in range(N_CORES):
        m = {"xq": q2[c * TOK:c * TOK + tok]}
        m.update(weights)
        in_maps.append(m)
    res = bass_utils.run_bass_kernel_spmd(
        nc, in_maps, core_ids=list(range(N_CORES)), **spmd_kwargs
    )
    outs = [res.results[c]["out"] for c in range(N_CORES)]
    return outs, res


def kernel(query, context, Wq, Wk, Wv, Wo, bo):
    outs, _ = run(query, Wq, Wk, Wv, Wo, bo)
    y = np.concatenate(outs, axis=0).reshape(B, N, D)
    bo = np.asarray(bo, np.float32)
    if bo.any():
        y = y + bo  # bias is structurally zero for this problem
    return y.astype(np.float32)


# revision 12
# speedup vs baseline: 1.6791x; 1.0638x over previous
"""Windowed local self-attention (CrossAttention with the context-overwrite
bug reproduced) on 8 Trainium2 NeuronCores — bf16 rewrite.

Full-input contract: kernel(**inputs) takes unsharded tensors, returns the
full (4, 4096, 1024) fp32 output. The 64 independent 256-token windows are
data-parallel sharded 8 per core; all four projection weights are broadcast.
No collectives.

Key structure (vs the fp32r baseline):
  * All matmul operands are bf16 (tolerance 2e-2; measured ~1e-3). bf16
    enables compiler-automatic FWL (2x faster LDWEIGHTS, hidden by the PE's
    64-deep reorder window).
  * X^T is produced by the DMA XBAR transpose (16-bit dtype) straight from
    HBM — no PE transposes, no DVE fixup copies.
  * Windows are processed in pairs so the projection matmuls stream N=512
    moving columns per instruction (half the instruction count).
  * Attention uses PE tile_position concurrency: sim row-tiles the two heads
    of an o-tile (K=64 each at row groups 0/64), S (ones-matmul row sums) and
    AV col-tile the same two heads (M=64 at col groups 0/64). Auto-derived
    from base partitions.
  * Software pipeline over pairs: iteration p issues proj(p) interleaved
    with attention(p-1) and the output projection Y(p-2), keeping the PE
    dense (HAM stays at K=8/8) while exp() runs on the ACT engine.

Per-pair layouts (i = pair-local token 0..511, ws = i//256):
  xt  [128, 8*512]  bf16  block kt: X^T[d in kt-tile, i]
  qT  [128, 8*512]  bf16  block ot: (XWq)^T[o in ot-tile, i]   (kT same)
  v   [128, 4*1024] bf16  block jj=(ws,jt): V[j in jj-tile, o]
  o2T [128, 8*512]  bf16  block ot: normalized attention output^T
"""

import numpy as np
import ml_dtypes

import concourse.bass as bass
import concourse.mybir as mybir
import concourse.tile as tile
from concourse import bacc, bass_utils
from concourse.bass_interp import get_hw_module

H = 16
DH = 64
WIN = 256
D = 1024
B = 4
N = 4096
N_CORES = 8
N_WIN_TOTAL = B * N // WIN          # 64
N_WIN = N_WIN_TOTAL // N_CORES      # 8 windows per core
N_PAIR = N_WIN // 2                 # 4 window-pairs per core
TOK = N_WIN * WIN                   # 2048 token rows per core
PTOK = 2 * WIN                      # 512 tokens per pair
SCALE = DH ** -0.5

F32 = mybir.dt.float32
BF16 = mybir.dt.bfloat16


def _body(tc, xq, wq, wk, wv, wo, out, n_pair):
    nc = tc.nc
    from contextlib import ExitStack

    with ExitStack() as ctx:
        singles = ctx.enter_context(tc.tile_pool(name="singles", bufs=1))
        xpool = ctx.enter_context(tc.tile_pool(name="xpool", bufs=2))
        acts = ctx.enter_context(tc.tile_pool(name="acts", bufs=2))
        heads = ctx.enter_context(tc.tile_pool(name="heads", bufs=4))
        psP = ctx.enter_context(tc.tile_pool(name="psP", bufs=3, space="PSUM"))
        psS = ctx.enter_context(tc.tile_pool(name="psS", bufs=2, space="PSUM"))
        psV = ctx.enter_context(tc.tile_pool(name="psV", bufs=2, space="PSUM"))

        # ---- weights on the scalar HWDGE queue so they land while the
        # sync queue streams xt(0); both gate the first qkT unit
        wsb = {}
        for name, w in (("wq", wq), ("wk", wk), ("wv", wv), ("wo", wo)):
            t = singles.tile([128, 8 * D], BF16, tag=name, name=f"sb_{name}")
            for kt in range(8):
                nc.scalar.dma_start(
                    t[:, kt * D:(kt + 1) * D], w[kt * 128:(kt + 1) * 128, :]
                )
            wsb[name] = t

        # xq is pre-transposed per pair on the host: row p*1024 + o holds
        # X^T[o, i] for pair p (o = global d, i = pair-local token)
        def alloc_xt(p):
            t = xpool.tile([128, 8 * PTOK], BF16, tag="xt", name=f"xt_{p}")
            for kt in range(8):
                r0 = p * D + kt * 128
                nc.sync.dma_start(
                    t[:, kt * PTOK:(kt + 1) * PTOK], xq[r0:r0 + 128, :]
                )
            return t

        xt_cur = alloc_xt(0)

        ones_f = singles.tile([128, 64], F32)
        nc.gpsimd.memset(ones_f[:], 1.0)
        ones64 = singles.tile([128, 64], BF16)
        nc.vector.tensor_copy(ones64[:], ones_f[:])

        # ================= emission helpers =================

        def emit_qkT_unit(p, dst, wname, ot):
            pq = psP.tile([128, PTOK], F32, tag="acc", name=f"pq_{p}_{wname}_{ot}")
            wtile = wsb[wname]
            for kt in range(8):
                nc.tensor.matmul(
                    pq[:],
                    wtile[:, kt * D + ot * 128:kt * D + (ot + 1) * 128],
                    xts[p][:, kt * PTOK:(kt + 1) * PTOK],
                    start=(kt == 0),
                    stop=(kt == 7),
                )
            nc.vector.tensor_copy(dst[:, ot * PTOK:(ot + 1) * PTOK], pq[:])

        def emit_v_unit(p, v_sb, jj, oc):
            pv = psP.tile([128, 512], F32, tag="acc", name=f"pv_{p}_{jj}_{oc}")
            for kt in range(8):
                nc.tensor.matmul(
                    pv[:],
                    xts[p][:, kt * PTOK + jj * 128:kt * PTOK + (jj + 1) * 128],
                    wsb["wv"][:, kt * D + oc * 512:kt * D + (oc + 1) * 512],
                    start=(kt == 0),
                    stop=(kt == 7),
                )
            nc.vector.tensor_copy(v_sb[:, jj * D + oc * 512:jj * D + (oc + 1) * 512], pv[:])

        def emit_sim(st, qT, kT, ws, pp):
            """sim for head pair (2pp, 2pp+1) of window ws; row-tiled."""
            col0 = pp * PTOK + ws * WIN
            es_pair = []
            ps_pair = []
            for hh in range(2):          # hh = h % 2, row group hh*64
                r0 = hh * 64
                ps_h = psS.tile([128, 512], F32, tag="sim",
                                name=f"sim_{st}_{hh}")
                ps_pair.append(ps_h)
            for jt in range(2):
                for hh in range(2):
                    r0 = hh * 64
                    nc.tensor.matmul(
                        ps_pair[hh][:, jt * WIN:(jt + 1) * WIN],
                        kT[r0:r0 + 64, col0 + jt * 128:col0 + (jt + 1) * 128],
                        qT[r0:r0 + 64, col0:col0 + WIN],
                        start=True,
                        stop=True,
                    )
            for hh in range(2):
                e = heads.tile([128, 512], BF16, tag="es", name=f"es_{st}_{hh}")
                nc.scalar.activation(
                    e[:], ps_pair[hh][:], mybir.ActivationFunctionType.Exp,
                    scale=SCALE,
                )
                es_pair.append(e)
            return es_pair

        def emit_sav(st, es_pair, v_sb, o2T, ws, pp):
            """row-sum S + AV for head pair; col-tiled; writes o2T block."""
            col0 = pp * PTOK + ws * WIN
            sav = psV.tile([128, 2 * WIN], F32, tag="sav", name=f"sav_{st}")
            s_ps = sav[:, 0:WIN]
            av_ps = sav[:, WIN:2 * WIN]
            for jt in range(2):
                for hh in range(2):
                    nc.tensor.matmul(
                        s_ps[hh * 64:(hh + 1) * 64, :],
                        ones64[:],
                        es_pair[hh][:, jt * WIN:(jt + 1) * WIN],
                        start=(jt == 0),
                        stop=(jt == 1),
                        skip_group_check=True,
                    )
            for jt in range(2):
                jj = ws * 2 + jt
                for hh in range(2):
                    h = 2 * pp + hh
                    nc.tensor.matmul(
                        av_ps[hh * 64:(hh + 1) * 64, :],
                        v_sb[:, jj * D + h * DH:jj * D + (h + 1) * DH],
                        es_pair[hh][:, jt * WIN:(jt + 1) * WIN],
                        start=(jt == 0),
                        stop=(jt == 1),
                        skip_group_check=True,
                    )
            rs = heads.tile([128, WIN], F32, tag="rs", name=f"rs_{st}", bufs=2)
            nc.vector.reciprocal_approx_fast(rs[:], s_ps[:])
            nc.vector.tensor_mul(o2T[:, col0:col0 + WIN], av_ps[:], rs[:])

        def emit_y_group(p, o2T, it, ec):
            py = psP.tile([128, 512], F32, tag="acc", name=f"py_{p}_{it}_{ec}")
            for ot in range(8):
                nc.tensor.matmul(
                    py[:],
                    o2T[:, ot * PTOK + it * 128:ot * PTOK + (it + 1) * 128],
                    wsb["wo"][:, ot * D + ec * 512:ot * D + (ec + 1) * 512],
                    start=(ot == 0),
                    stop=(ot == 7),
                )
            y_sb = acts.tile([128, 512], F32, tag="y", name=f"y_{p}_{it}_{ec}", bufs=3)
            nc.vector.tensor_copy(y_sb[:], py[:])
            r0 = p * PTOK + it * 128
            nc.sync.dma_start(out[r0:r0 + 128, ec * 512:(ec + 1) * 512], y_sb[:])

        # ================= software pipeline =================

        xts = {0: xt_cur}
        pair_tiles = {}   # p -> (qT, kT, v_sb, o2T)
        ATT_STEPS = [(ws, pp) for ws in range(2) for pp in range(8)]

        def attn_step_sim(p, st):
            ws, pp = ATT_STEPS[st]
            qT, kT, v_sb, o2T = pair_tiles[p]
            return emit_sim(f"{p}_{st}", qT, kT, ws, pp)

        def attn_step_sav(p, st, es_pair):
            ws, pp = ATT_STEPS[st]
            qT, kT, v_sb, o2T = pair_tiles[p]
            emit_sav(f"{p}_{st}", es_pair, v_sb, o2T, ws, pp)

        for p in range(n_pair):
            if p + 1 < n_pair:
                xts[p + 1] = alloc_xt(p + 1)

            qT = acts.tile([128, 8 * PTOK], BF16, tag="qT", name=f"qT_{p}")
            kT = acts.tile([128, 8 * PTOK], BF16, tag="kT", name=f"kT_{p}")
            v_sb = acts.tile([128, 4 * D], BF16, tag="v", name=f"v_{p}")
            o2T = acts.tile([128, 8 * PTOK], BF16, tag="o2T", name=f"o2T_{p}")
            pair_tiles[p] = (qT, kT, v_sb, o2T)

            proj_units = (
                [lambda ot=ot: emit_qkT_unit(p, qT, "wq", ot) for ot in range(8)]
                + [lambda ot=ot: emit_qkT_unit(p, kT, "wk", ot) for ot in range(8)]
                + [lambda jj=jj, oc=oc: emit_v_unit(p, v_sb, jj, oc)
                   for jj in range(4) for oc in range(2)]
            )
            y_units = []
            if p >= 2:
                o2T_y = pair_tiles[p - 2][3]
                y_units = [
                    lambda it=it, ec=ec: emit_y_group(p - 2, o2T_y, it, ec)
                    for it in range(4) for ec in range(2)
                ]

            if p == 0:
                for u in proj_units:
                    u()
            else:
                # interleave: 16 attention steps of pair p-1, one-step
                # sim->sav delay, filler = proj units of p + Y units of p-2
                filler = proj_units + y_units
                fi = 0
                pending = None
                for st in range(16):
                    es_pair = attn_step_sim(p - 1, st)
                    if fi < len(filler):
                        filler[fi](); fi += 1
                    if pending is not None:
                        attn_step_sav(p - 1, pending[0], pending[1])
                    pending = (st, es_pair)
                    if fi < len(filler):
                        filler[fi](); fi += 1
                attn_step_sav(p - 1, pending[0], pending[1])
                while fi < len(filler):
                    filler[fi](); fi += 1

        # ---- drain: attention of last pair + Y of pairs n-2, n-1 ----
        last = n_pair - 1
        filler = []
        if n_pair >= 2:
            o2T_y = pair_tiles[n_pair - 2][3]
            filler = [
                lambda it=it, ec=ec: emit_y_group(n_pair - 2, o2T_y, it, ec)
                for it in range(4) for ec in range(2)
            ]
        fi = 0
        pending = None
        late_y = []
        o2T_last = pair_tiles[last][3]
        for st in range(16):
            es_pair = attn_step_sim(last, st)
            if fi < len(filler):
                filler[fi](); fi += 1
            if pending is not None:
                attn_step_sav(last, pending[0], pending[1])
            pending = (st, es_pair)
            if st == 9:
                # window 0 of the last pair fully normalized after step 7's
                # sav (emitted during st=8); its Y groups are ready now
                late_y = [
                    lambda it=it, ec=ec: emit_y_group(last, o2T_last, it, ec)
                    for it in range(2) for ec in range(2)
                ]
            if late_y:
                late_y.pop(0)()
            elif fi < len(filler):
                filler[fi](); fi += 1
        attn_step_sav(last, pending[0], pending[1])
        while fi < len(filler):
            filler[fi](); fi += 1
        for it in range(2, 4):
            for ec in range(2):
                emit_y_group(last, o2T_last, it, ec)


_CACHE = {}


def _build(n_pair=N_PAIR):
    key = n_pair
    if key in _CACHE:
        return _CACHE[key]
    tok = n_pair * PTOK
    nc = bacc.Bacc(
        "TRN2", target_bir_lowering=False, debug=False, num_devices=N_CORES
    )
    xq = nc.dram_tensor("xq", [n_pair * D, PTOK], BF16, kind="ExternalInput").ap()
    wq = nc.dram_tensor("Wq", [D, D], BF16, kind="ExternalInput").ap()
    wk = nc.dram_tensor("Wk", [D, D], BF16, kind="ExternalInput").ap()
    wv = nc.dram_tensor("Wv", [D, D], BF16, kind="ExternalInput").ap()
    wo = nc.dram_tensor("Wo", [D, D], BF16, kind="ExternalInput").ap()
    out = nc.dram_tensor("out", [tok, D], F32, kind="ExternalOutput").ap()
    with tile.TileContext(nc) as tc:
        _body(tc, xq, wq, wk, wv, wo, out, n_pair)
    nc.compile()
    nc.m = get_hw_module(nc.m)
    _CACHE[key] = nc
    return nc


def run(query, Wq, Wk, Wv, Wo, bo, n_pair=N_PAIR, **spmd_kwargs):
    nc = _build(n_pair)
    tok = n_pair * PTOK
    # shard prep: bf16 cast + per-pair transpose so each core streams
    # X^T[d, pair-token] with contiguous 1KB DMA lines
    q2 = (
        np.asarray(query, dtype=np.float32)
        .astype(ml_dtypes.bfloat16)
        .reshape(-1, PTOK, D)
        .transpose(0, 2, 1)      # [n_pair_total, D, PTOK]
        .reshape(-1, PTOK)       # rows: pair-major, then d
    )
    q2 = np.ascontiguousarray(q2)
    weights = {
        "Wq": np.asarray(Wq, np.float32).astype(ml_dtypes.bfloat16),
        "Wk": np.asarray(Wk, np.float32).astype(ml_dtypes.bfloat16),
        "Wv": np.asarray(Wv, np.float32).astype(ml_dtypes.bfloat16),
        "Wo": np.asarray(Wo, np.float32).astype(ml_dtypes.bfloat16),
    }
    in_maps = []
    rows_per_core = N_PAIR * D
    for c in range(N_CORES):
        m = {"xq": q2[c * rows_per_core:c * rows_per_core + n_pair * D]}
        m.update(weights)
        in_maps.append(m)
    res = bass_utils.run_bass_kernel_spmd(
        nc, in_maps, core_ids=list(range(N_CORES)), **spmd_kwargs
    )
    outs = [res.results[c]["out"] for c in range(N_CORES)]
    return outs, res


def kernel(query, context, Wq, Wk, Wv, Wo, bo):
    outs, _ = run(query, Wq, Wk, Wv, Wo, bo)
    y = np.concatenate(outs, axis=0).reshape(B, N, D)
    bo = np.asarray(bo, np.float32)
    if bo.any():
        y = y + bo  # bias is structurally zero for this problem
    return y.astype(np.float32)


# revision 15
# speedup vs baseline: 1.6870x; 1.0047x over previous
"""Windowed local self-attention (CrossAttention with the context-overwrite
bug reproduced) on 8 Trainium2 NeuronCores — bf16 rewrite.

Full-input contract: kernel(**inputs) takes unsharded tensors, returns the
full (4, 4096, 1024) fp32 output. The 64 independent 256-token windows are
data-parallel sharded 8 per core; all four projection weights are broadcast.
No collectives.

Key structure (vs the fp32r baseline):
  * All matmul operands are bf16 (tolerance 2e-2; measured ~1e-3). bf16
    enables compiler-automatic FWL (2x faster LDWEIGHTS, hidden by the PE's
    64-deep reorder window).
  * X^T is produced by the DMA XBAR transpose (16-bit dtype) straight from
    HBM — no PE transposes, no DVE fixup copies.
  * Windows are processed in pairs so the projection matmuls stream N=512
    moving columns per instruction (half the instruction count).
  * Attention uses PE tile_position concurrency: sim row-tiles the two heads
    of an o-tile (K=64 each at row groups 0/64), S (ones-matmul row sums) and
    AV col-tile the same two heads (M=64 at col groups 0/64). Auto-derived
    from base partitions.
  * Software pipeline over pairs: iteration p issues proj(p) interleaved
    with attention(p-1) and the output projection Y(p-2), keeping the PE
    dense (HAM stays at K=8/8) while exp() runs on the ACT engine.

Per-pair layouts (i = pair-local token 0..511, ws = i//256):
  xt  [128, 8*512]  bf16  block kt: X^T[d in kt-tile, i]
  qT  [128, 8*512]  bf16  block ot: (XWq)^T[o in ot-tile, i]   (kT same)
  v   [128, 4*1024] bf16  block jj=(ws,jt): V[j in jj-tile, o]
  o2T [128, 8*512]  bf16  block ot: normalized attention output^T
"""

import numpy as np
import ml_dtypes

import concourse.bass as bass
import concourse.mybir as mybir
import concourse.tile as tile
from concourse import bacc, bass_utils
from concourse.bass_interp import get_hw_module

H = 16
DH = 64
WIN = 256
D = 1024
B = 4
N = 4096
N_CORES = 8
N_WIN_TOTAL = B * N // WIN          # 64
N_WIN = N_WIN_TOTAL // N_CORES      # 8 windows per core
N_PAIR = N_WIN // 2                 # 4 window-pairs per core
TOK = N_WIN * WIN                   # 2048 token rows per core
PTOK = 2 * WIN                      # 512 tokens per pair
SCALE = DH ** -0.5

F32 = mybir.dt.float32
BF16 = mybir.dt.bfloat16


def _body(tc, xq, wq, wk, wv, wo, out, n_pair):
    nc = tc.nc
    from contextlib import ExitStack

    with ExitStack() as ctx:
        singles = ctx.enter_context(tc.tile_pool(name="singles", bufs=1))
        xpool = ctx.enter_context(tc.tile_pool(name="xpool", bufs=2))
        acts = ctx.enter_context(tc.tile_pool(name="acts", bufs=2))
        heads = ctx.enter_context(tc.tile_pool(name="heads", bufs=4))
        psP = ctx.enter_context(tc.tile_pool(name="psP", bufs=3, space="PSUM"))
        psS = ctx.enter_context(tc.tile_pool(name="psS", bufs=2, space="PSUM"))
        psV = ctx.enter_context(tc.tile_pool(name="psV", bufs=3, space="PSUM"))

        # ---- weights on the scalar HWDGE queue so they land while the
        # sync queue streams xt(0); both gate the first qkT unit
        wsb = {}
        for name, w in (("wq", wq), ("wk", wk), ("wv", wv), ("wo", wo)):
            t = singles.tile([128, 8 * D], BF16, tag=name, name=f"sb_{name}")
            for kt in range(8):
                nc.scalar.dma_start(
                    t[:, kt * D:(kt + 1) * D], w[kt * 128:(kt + 1) * 128, :]
                )
            wsb[name] = t

        # xq is pre-transposed per pair on the host: row p*1024 + o holds
        # X^T[o, i] for pair p (o = global d, i = pair-local token)
        def alloc_xt(p):
            t = xpool.tile([128, 8 * PTOK], BF16, tag="xt", name=f"xt_{p}")
            for kt in range(8):
                r0 = p * D + kt * 128
                nc.sync.dma_start(
                    t[:, kt * PTOK:(kt + 1) * PTOK], xq[r0:r0 + 128, :]
                )
            return t

        xt_cur = alloc_xt(0)

        ones_f = singles.tile([128, 64], F32)
        nc.gpsimd.memset(ones_f[:], 1.0)
        ones64 = singles.tile([128, 64], BF16)
        nc.vector.tensor_copy(ones64[:], ones_f[:])

        # ================= emission helpers =================

        def emit_qkT_unit(p, dst, wname, ot):
            pq = psP.tile([128, PTOK], F32, tag="acc", name=f"pq_{p}_{wname}_{ot}")
            wtile = wsb[wname]
            for kt in range(8):
                nc.tensor.matmul(
                    pq[:],
                    wtile[:, kt * D + ot * 128:kt * D + (ot + 1) * 128],
                    xts[p][:, kt * PTOK:(kt + 1) * PTOK],
                    start=(kt == 0),
                    stop=(kt == 7),
                )
            nc.vector.tensor_copy(dst[:, ot * PTOK:(ot + 1) * PTOK], pq[:])

        def emit_v_unit(p, v_sb, jj, oc):
            pv = psP.tile([128, 512], F32, tag="acc", name=f"pv_{p}_{jj}_{oc}")
            for kt in range(8):
                nc.tensor.matmul(
                    pv[:],
                    xts[p][:, kt * PTOK + jj * 128:kt * PTOK + (jj + 1) * 128],
                    wsb["wv"][:, kt * D + oc * 512:kt * D + (oc + 1) * 512],
                    start=(kt == 0),
                    stop=(kt == 7),
                )
            nc.vector.tensor_copy(v_sb[:, jj * D + oc * 512:jj * D + (oc + 1) * 512], pv[:])

        def emit_sim(st, qT, kT, ws, pp):
            """sim for head pair (2pp, 2pp+1) of window ws; row-tiled."""
            col0 = pp * PTOK + ws * WIN
            es_pair = []
            ps_pair = []
            for hh in range(2):          # hh = h % 2, row group hh*64
                r0 = hh * 64
                ps_h = psS.tile([128, 512], F32, tag="sim",
                                name=f"sim_{st}_{hh}")
                ps_pair.append(ps_h)
            for jt in range(2):
                for hh in range(2):
                    r0 = hh * 64
                    nc.tensor.matmul(
                        ps_pair[hh][:, jt * WIN:(jt + 1) * WIN],
                        kT[r0:r0 + 64, col0 + jt * 128:col0 + (jt + 1) * 128],
                        qT[r0:r0 + 64, col0:col0 + WIN],
                        start=True,
                        stop=True,
                    )
            for hh in range(2):
                e = heads.tile([128, 512], BF16, tag="es", name=f"es_{st}_{hh}")
                nc.scalar.activation(
                    e[:], ps_pair[hh][:], mybir.ActivationFunctionType.Exp,
                    scale=SCALE,
                )
                es_pair.append(e)
            return es_pair

        def emit_sav(st, es_pair, v_sb, o2T, ws, pp):
            """row-sum S + AV for head pair; col-tiled; writes o2T block."""
            col0 = pp * PTOK + ws * WIN
            sav = psV.tile([128, 2 * WIN], F32, tag="sav", name=f"sav_{st}")
            s_ps = sav[:, 0:WIN]
            av_ps = sav[:, WIN:2 * WIN]
            for jt in range(2):
                for hh in range(2):
                    nc.tensor.matmul(
                        s_ps[hh * 64:(hh + 1) * 64, :],
                        ones64[:],
                        es_pair[hh][:, jt * WIN:(jt + 1) * WIN],
                        start=(jt == 0),
                        stop=(jt == 1),
                        skip_group_check=True,
                    )
            for jt in range(2):
                jj = ws * 2 + jt
                for hh in range(2):
                    h = 2 * pp + hh
                    nc.tensor.matmul(
                        av_ps[hh * 64:(hh + 1) * 64, :],
                        v_sb[:, jj * D + h * DH:jj * D + (h + 1) * DH],
                        es_pair[hh][:, jt * WIN:(jt + 1) * WIN],
                        start=(jt == 0),
                        stop=(jt == 1),
                        skip_group_check=True,
                    )
            rs = heads.tile([128, WIN], F32, tag="rs", name=f"rs_{st}", bufs=2)
            nc.vector.reciprocal_approx_fast(rs[:], s_ps[:])
            nc.vector.tensor_mul(o2T[:, col0:col0 + WIN], av_ps[:], rs[:])

        def emit_y_group(p, o2T, it, ec):
            py = psP.tile([128, 512], F32, tag="acc", name=f"py_{p}_{it}_{ec}")
            for ot in range(8):
                nc.tensor.matmul(
                    py[:],
                    o2T[:, ot * PTOK + it * 128:ot * PTOK + (it + 1) * 128],
                    wsb["wo"][:, ot * D + ec * 512:ot * D + (ec + 1) * 512],
                    start=(ot == 0),
                    stop=(ot == 7),
                )
            y_sb = acts.tile([128, 512], F32, tag="y", name=f"y_{p}_{it}_{ec}", bufs=3)
            nc.vector.tensor_copy(y_sb[:], py[:])
            r0 = p * PTOK + it * 128
            nc.sync.dma_start(out[r0:r0 + 128, ec * 512:(ec + 1) * 512], y_sb[:])

        # ================= software pipeline =================

        xts = {0: xt_cur}
        pair_tiles = {}   # p -> (qT, kT, v_sb, o2T)
        ATT_STEPS = [(ws, pp) for ws in range(2) for pp in range(8)]

        def attn_step_sim(p, st):
            ws, pp = ATT_STEPS[st]
            qT, kT, v_sb, o2T = pair_tiles[p]
            return emit_sim(f"{p}_{st}", qT, kT, ws, pp)

        def attn_step_sav(p, st, es_pair):
            ws, pp = ATT_STEPS[st]
            qT, kT, v_sb, o2T = pair_tiles[p]
            emit_sav(f"{p}_{st}", es_pair, v_sb, o2T, ws, pp)

        for p in range(n_pair):
            if p + 1 < n_pair:
                xts[p + 1] = alloc_xt(p + 1)

            qT = acts.tile([128, 8 * PTOK], BF16, tag="qT", name=f"qT_{p}")
            kT = acts.tile([128, 8 * PTOK], BF16, tag="kT", name=f"kT_{p}")
            v_sb = acts.tile([128, 4 * D], BF16, tag="v", name=f"v_{p}")
            o2T = acts.tile([128, 8 * PTOK], BF16, tag="o2T", name=f"o2T_{p}")
            pair_tiles[p] = (qT, kT, v_sb, o2T)

            proj_units = (
                [lambda ot=ot: emit_qkT_unit(p, qT, "wq", ot) for ot in range(8)]
                + [lambda ot=ot: emit_qkT_unit(p, kT, "wk", ot) for ot in range(8)]
                + [lambda jj=jj, oc=oc: emit_v_unit(p, v_sb, jj, oc)
                   for jj in range(4) for oc in range(2)]
            )
            y_units = []
            if p >= 2:
                o2T_y = pair_tiles[p - 2][3]
                y_units = [
                    lambda it=it, ec=ec: emit_y_group(p - 2, o2T_y, it, ec)
                    for it in range(4) for ec in range(2)
                ]

            if p == 0:
                for u in proj_units:
                    u()
            else:
                # interleave: 16 attention steps of pair p-1, one-step
                # sim->sav delay, filler = proj units of p + Y units of p-2.
                # [sim, sav, F, F] order: attention MMs stay adjacent, so the
                # full-array LDWEIGHTS stall at each attn<->proj transition is
                # paid twice per step instead of four times.
                filler = proj_units + y_units
                fi = 0
                pending = None
                for st in range(16):
                    es_pair = attn_step_sim(p - 1, st)
                    if pending is not None:
                        attn_step_sav(p - 1, pending[0], pending[1])
                    pending = (st, es_pair)
                    for _ in range(2):
                        if fi < len(filler):
                            filler[fi](); fi += 1
                attn_step_sav(p - 1, pending[0], pending[1])
                while fi < len(filler):
                    filler[fi](); fi += 1

        # ---- drain: attention of last pair + Y of pairs n-2, n-1 ----
        last = n_pair - 1
        filler = []
        if n_pair >= 2:
            o2T_y = pair_tiles[n_pair - 2][3]
            filler = [
                lambda it=it, ec=ec: emit_y_group(n_pair - 2, o2T_y, it, ec)
                for it in range(4) for ec in range(2)
            ]
        fi = 0
        pending = None
        late_y = []
        o2T_last = pair_tiles[last][3]
        for st in range(16):
            es_pair = attn_step_sim(last, st)
            if pending is not None:
                attn_step_sav(last, pending[0], pending[1])
            pending = (st, es_pair)
            if st == 9:
                # window 0 of the last pair fully normalized after step 7's
                # sav (emitted during st=8); its Y groups are ready now
                late_y = [
                    lambda it=it, ec=ec: emit_y_group(last, o2T_last, it, ec)
                    for it in range(2) for ec in range(2)
                ]
            for _ in range(2):
                if late_y:
                    late_y.pop(0)()
                elif fi < len(filler):
                    filler[fi](); fi += 1
        attn_step_sav(last, pending[0], pending[1])
        while fi < len(filler):
            filler[fi](); fi += 1
        for it in range(2, 4):
            for ec in range(2):
                emit_y_group(last, o2T_last, it, ec)


_CACHE = {}


def _build(n_pair=N_PAIR):
    key = n_pair
    if key in _CACHE:
        return _CACHE[key]
    tok = n_pair * PTOK
    nc = bacc.Bacc(
        "TRN2", target_bir_lowering=False, debug=False, num_devices=N_CORES
    )
    xq = nc.dram_tensor("xq", [n_pair * D, PTOK], BF16, kind="ExternalInput").ap()
    wq = nc.dram_tensor("Wq", [D, D], BF16, kind="ExternalInput").ap()
    wk = nc.dram_tensor("Wk", [D, D], BF16, kind="ExternalInput").ap()
    wv = nc.dram_tensor("Wv", [D, D], BF16, kind="ExternalInput").ap()
    wo = nc.dram_tensor("Wo", [D, D], BF16, kind="ExternalInput").ap()
    out = nc.dram_tensor("out", [tok, D], F32, kind="ExternalOutput").ap()
    with tile.TileContext(nc) as tc:
        _body(tc, xq, wq, wk, wv, wo, out, n_pair)
    nc.compile()
    nc.m = get_hw_module(nc.m)
    _CACHE[key] = nc
    return nc


def run(query, Wq, Wk, Wv, Wo, bo, n_pair=N_PAIR, **spmd_kwargs):
    nc = _build(n_pair)
    tok = n_pair * PTOK
    # shard prep: bf16 cast + per-pair transpose so each core streams
    # X^T[d, pair-token] with contiguous 1KB DMA lines
    q2 = (
        np.asarray(query, dtype=np.float32)
        .astype(ml_dtypes.bfloat16)
        .reshape(-1, PTOK, D)
        .transpose(0, 2, 1)      # [n_pair_total, D, PTOK]
        .reshape(-1, PTOK)       # rows: pair-major, then d
    )
    q2 = np.ascontiguousarray(q2)
    weights = {
        "Wq": np.asarray(Wq, np.float32).astype(ml_dtypes.bfloat16),
        "Wk": np.asarray(Wk, np.float32).astype(ml_dtypes.bfloat16),
        "Wv": np.asarray(Wv, np.float32).astype(ml_dtypes.bfloat16),
        "Wo": np.asarray(Wo, np.float32).astype(ml_dtypes.bfloat16),
    }
    in_maps = []
    rows_per_core = N_PAIR * D
    for c in range(N_CORES):
        m = {"xq": q2[c * rows_per_core:c * rows_per_core + n_pair * D]}
        m.update(weights)
        in_maps.append(m)
    res = bass_utils.run_bass_kernel_spmd(
        nc, in_maps, core_ids=list(range(N_CORES)), **spmd_kwargs
    )
    outs = [res.results[c]["out"] for c in range(N_CORES)]
    return outs, res


def kernel(query, context, Wq, Wk, Wv, Wo, bo):
    outs, _ = run(query, Wq, Wk, Wv, Wo, bo)
    y = np.concatenate(outs, axis=0).reshape(B, N, D)
    bo = np.asarray(bo, np.float32)
    if bo.any():
        y = y + bo  # bias is structurally zero for this problem
    return y.astype(np.float32)
